# revision 1
# baseline (speedup 1.0000x reference)
"""Trainium2 Bass kernel for nn_Blast: out = x @ (W0 + 1 bias^T) + bias
where W0 block (i_in, i_out) = Vt[i] @ diag(S[o,i]) @ U[o].

Factorized algorithm (per core, 256 tokens):
  midT[(o,r), tok] = sum_in A[in, (o,r)] * xT[in, tok]     (A = Vt*S, built on device)
  out[tok, oq]     = sum_r midT[(o,r), tok] * U''[o, r, q]

Layout: the 272 mid rows (16 o-blocks x 17) live at 32-aligned slots
(o -> psum group g=o//4, slot j=o%4, rows 32j..32j+16); A is zero-padded to
512 columns so the A-phase runs full-128-row matmuls (f32r forbids PE
subarray tiling, and only full-K matmul streams engage the PE's 2.4 GHz
activity monitor).

Bias trick: out = x@W0 + (rowsum(x)+1)*bias.  A has a 17th all-ones column
per o-block (-> rowsum in mid row 32j+16); each mid bank is opened by a
matmul writing 1.0 everywhere, so rank rows carry mid+1 and padding rows
carry 1.0; U'' row 16 = bias (multiplies rowsum+1), row 17 = -sum_r U[o,r]
(cancels the +1 pollution via the 1.0 padding row). U'' is zero-padded to
K=128 so the B-phase matmuls also run full-K (stay warm) and share one
weight load per group of four output blocks.

PE warmup: ~40 dummy full-K matmuls run during the input-DMA window; the
hardware activity monitor only unthrottles 1.2->2.4 GHz after ~a window of
contiguous full-K matmul activity, and low-K matmuls do not count.

Sharding: pure data-parallel over the 2048 tokens (8 cores x 256); the
small factors are replicated. x is fed pre-transposed (xT) from the host.
"""

import numpy as np

IN_DIM = 4096
OUT_DIM = 4096
BLOCK = 256
RANK = 16
B_IN = 16
B_OUT = 16
N_CORES = 8
TOK = 2048
TPC = TOK // N_CORES          # 256 tokens per core
RA = RANK + 1                 # 17: rank cols + rowsum col per o-block
KU = RANK + 2                 # 18: used rows of U'' per o-block
CP = 32                       # padded per-o column stride (32-aligned slots)
CAP = B_OUT * CP              # 512 padded columns of A
NCHUNK = IN_DIM // 128        # 32 K-chunks
NWARM = 28                    # PE warmup matmuls

_CACHE = {}

# test.py toggles; harness never touches these
TRACE = False
TRACE_DIR = None
LAST_RESULTS = None


def build_program():
    import concourse.mybir as mybir
    from concourse import bacc
    from concourse.tile import TileContext

    f32 = mybir.dt.float32
    f32r = mybir.dt.float32r

    nc = bacc.Bacc(trn_type="TRN2")
    xt_d = nc.dram_tensor("xt", (IN_DIM, TPC), f32r, kind="ExternalInput")
    vt_d = nc.dram_tensor("vt", (B_IN, BLOCK, CP), f32, kind="ExternalInput")
    s_d = nc.dram_tensor("s_flat", (1, B_IN * CAP), f32r, kind="ExternalInput")
    aship_d = nc.dram_tensor("aship", (B_IN // 2, 2 * 128, CAP), f32r, kind="ExternalInput")
    u_d = nc.dram_tensor("u_mat", (B_OUT, KU, BLOCK), f32r, kind="ExternalInput")
    w_d = nc.dram_tensor("wseed", (128, BLOCK), f32r, kind="ExternalInput")
    konst_d = nc.dram_tensor("konst", (1, 2 * TPC), f32r, kind="ExternalInput")
    out_d = nc.dram_tensor("out", (TPC, OUT_DIM), f32, kind="ExternalOutput")

    with TileContext(nc) as tc:
        from contextlib import ExitStack

        with ExitStack() as ctx:
            consts = ctx.enter_context(tc.tile_pool(name="consts", bufs=1))
            spool = ctx.enter_context(tc.tile_pool(name="spool", bufs=4))
            xpool = ctx.enter_context(tc.tile_pool(name="xpool", bufs=1))
            apool = ctx.enter_context(tc.tile_pool(name="apool", bufs=1))
            midsb = ctx.enter_context(tc.tile_pool(name="midsb", bufs=1))
            outsb = ctx.enter_context(tc.tile_pool(name="outsb", bufs=6))
            ps_mid = ctx.enter_context(
                tc.tile_pool(name="ps_mid", bufs=1, space="PSUM")
            )

            # ---- input loads ----
            # warm-up seed: first transfer on the sync queue
            wsb = consts.tile([128, BLOCK], f32r, name="wsb", tag="wsb")
            nc.sync.dma_start(out=wsb[:], in_=w_d[:])

            # memset can't produce f32r (ISA), so ones come via DMA:
            # konst = [ones(256) | zeros(256)]
            konst_sb = consts.tile([1, 2 * TPC], f32r, name="konst_sb", tag="konst_sb")
            nc.gpsimd.dma_start(out=konst_sb[:], in_=konst_d[:])
            ones_sb = konst_sb[0:1, 0:128]
            onestpc_sb = konst_sb[0:1, 0:TPC]

            s_sb = consts.tile([1, B_IN * CAP], f32r, name="s_sb", tag="s_sb")
            nc.gpsimd.dma_start(out=s_sb[:], in_=s_d[:])

            # all Vt chunks in one DMA: vt_all[p, i, h, r], h = 128-row half
            vt_all = consts.tile([128, B_IN * 2 * CP], f32, name="vt_all", tag="vt_all")
            nc.gpsimd.dma_start(
                out=vt_all[:].rearrange("p (i a r) -> p i a r", i=B_IN, a=2),
                in_=vt_d[:].rearrange("i (a p) r -> p i a r", p=128),
            )
            vt_v = vt_all[:].rearrange("p (i a r) -> p i a r", i=B_IN, a=2)

            # U'': usb[32*(o%4)+r, o*256+q] = U''[o,r,q]; one DMA per slot j
            usb = consts.tile([128, B_OUT * BLOCK], f32r, name="usb", tag="usb")
            for j in range(4):
                nc.gpsimd.dma_start(
                    out=usb[32 * j : 32 * j + KU, :]
                    .rearrange("r (g q) -> r g q", g=4)[:, :, j * BLOCK : (j + 1) * BLOCK],
                    in_=u_d[:].rearrange("(g jj) r q -> jj r g q", jj=4)[j],
                )

            # x^T chunk batches interleaved with shipped A chunks (even i)
            # on the sync queue; chunks for odd i are built on device below
            XGRP = 4
            xbatches = []
            ashipped = {}
            for b in range(NCHUNK // XGRP):
                xb = xpool.tile([128, XGRP * TPC], f32r, name=f"xb{b}", tag=f"xb{b}")
                nc.sync.dma_start(
                    out=xb[:].rearrange("p (k t) -> p k t", k=XGRP),
                    in_=xt_d[b * XGRP * 128 : (b + 1) * XGRP * 128, :].rearrange(
                        "(k p) t -> p k t", p=128
                    ),
                )
                xbatches.append(xb)
                i = 2 * b  # even i whose chunk pair ships whole
                if i < B_IN:
                    ab = apool.tile(
                        [128, 2 * CAP], f32r, name=f"ab{i}", tag=f"ab{i}"
                    )
                    # early pairs ride the sync ring between x batches; late
                    # pairs go via the GpSimd queue so the x tail isn't
                    # serialized behind them
                    eng = nc.sync if i <= 4 else nc.gpsimd
                    eng.dma_start(
                        out=ab[:].rearrange("p (two c) -> p two c", two=2),
                        in_=aship_d[i // 2].rearrange("(two p) c -> p two c", p=128),
                    )
                    ashipped[2 * i] = ab[:, 0:CAP]
                    ashipped[2 * i + 1] = ab[:, CAP : 2 * CAP]

            def xchunk(k):
                return xbatches[k // XGRP][:, (k % XGRP) * TPC : (k % XGRP + 1) * TPC]


            # ---- A-builds: S row broadcast (PE), stage (ACT), Vt*S (DVE/GPS)
            # These engines start as soon as s/vt land, overlapping the PE
            # warmup below; the A-phase then never waits on a build.
            midp = []
            abuilt = {}
            with tc.tile_pool(name="ps_pre", bufs=1, space="PSUM") as ps_pre:
                # ---- PE warmup while inputs stream in ----
                warm = ps_pre.tile([128, BLOCK], f32, name="warm", tag="warm", bufs=1)
                for w in range(NWARM):
                    nc.tensor.matmul(
                        warm[:],
                        lhsT=wsb[:, 0:128],
                        rhs=wsb[:],
                        start=True,
                        stop=True,
                        tile_position=(0, 0),
                    )

                for i in range(1, B_IN, 2):
                    sp = ps_pre.tile([128, CAP], f32, name="sp", tag="sp", bufs=3)
                    nc.tensor.matmul(
                        sp[:],
                        lhsT=ones_sb,
                        rhs=s_sb[0:1, i * CAP : (i + 1) * CAP],
                        start=True,
                        stop=True,
                        tile_position=(0, 0),
                    )
                    sps = spool.tile([128, CAP], f32, name="sps", tag="sps")
                    nc.scalar.copy(sps[:], sp[:])
                    for h in range(2):
                        k = 2 * i + h
                        a_t = apool.tile([128, CAP], f32r, name=f"a{k}", tag=f"a{k}")
                        eng = nc.gpsimd if i >= 13 else nc.vector
                        eng.tensor_mul(
                            a_t[:].rearrange("p (o r) -> p o r", r=CP),
                            vt_v[:, i, h, :]
                            .unsqueeze(1)
                            .broadcast_to([128, B_OUT, CP]),
                            sps[:].rearrange("p (o r) -> p o r", r=CP),
                        )
                        abuilt[k] = a_t

                # ---- open the mid banks with 1.0 everywhere ----
                for g in range(4):
                    mp = ps_mid.tile([128, TPC], f32, name=f"midp{g}", tag=f"midp{g}")
                    nc.tensor.matmul(
                        mp[:],
                        lhsT=ones_sb,
                        rhs=onestpc_sb,
                        start=True,
                        stop=False,
                        tile_position=(0, 0),
                    )
                    midp.append(mp)

                asbs = [
                    ashipped[k] if k in ashipped else abuilt[k]
                    for k in range(NCHUNK)
                ]

                # ---- phase A: midT accumulation over 32 K-chunks ----
                # a dummy warm matmul after every other chunk keeps the PE
                # activity monitor latched through DMA-starvation gaps
                for k in range(NCHUNK):
                    for g in range(4):
                        nc.tensor.matmul(
                            midp[g][:],
                            lhsT=asbs[k][:, g * 128 : (g + 1) * 128],
                            rhs=xchunk(k),
                            start=False,
                            stop=(k == NCHUNK - 1),
                            tile_position=(0, 0),
                        )
                    nfill = 2 if 6 <= k <= 24 else (1 if 2 <= k <= 26 else 0)
                    for _ in range(nfill):
                        nc.tensor.matmul(
                            warm[:],
                            lhsT=wsb[:, 0:128],
                            rhs=wsb[:],
                            start=True,
                            stop=True,
                            tile_position=(0, 0),
                        )

            # ---- midT to SBUF, one token-half at a time so phase B can
            # start on half 0 while half 1 still copies ----
            mids = []
            for g in range(4):
                ms = midsb.tile([128, TPC], f32r, name=f"mids{g}", tag=f"mids{g}")
                mids.append(ms)
            for tt in range(2):
                for g in range(4):
                    sl = (slice(None), slice(tt * 128, (tt + 1) * 128))
                    if (g + tt) % 2 == 0:
                        nc.scalar.copy(mids[g][sl], midp[g][sl])
                    else:
                        nc.vector.tensor_copy(mids[g][sl], midp[g][sl])

            # ---- phase B: out tiles [128 tok, 256 q], K=128 ----
            ps_out = ctx.enter_context(
                tc.tile_pool(name="ps_out", bufs=4, space="PSUM")
            )
            OGRP = 4  # o-blocks per output DMA; o//4 == g inside a group
            for tt in range(TPC // 128):
                for og in range(B_OUT // OGRP):
                    osb_t = outsb.tile(
                        [128, OGRP * BLOCK], f32, name="osb", tag="osb"
                    )
                    for oo in range(OGRP):
                        o = og * OGRP + oo
                        po = ps_out.tile([128, BLOCK], f32, name="po", tag="po")
                        j = o % 4
                        nc.tensor.matmul(
                            po[:],
                            lhsT=mids[o // 4][
                                32 * j : 32 * j + KU, tt * 128 : (tt + 1) * 128
                            ],
                            rhs=usb[
                                32 * j : 32 * j + KU, o * BLOCK : (o + 1) * BLOCK
                            ],
                            start=True,
                            stop=True,
                            tile_position=(32 * j, 0),
                        )
                        if o % 2 == 0:
                            nc.vector.tensor_copy(
                                osb_t[:, oo * BLOCK : (oo + 1) * BLOCK], po[:]
                            )
                        else:
                            nc.scalar.copy(
                                osb_t[:, oo * BLOCK : (oo + 1) * BLOCK], po[:]
                            )
                    nc.sync.dma_start(
                        out=out_d[
                            tt * 128 : (tt + 1) * 128,
                            og * OGRP * BLOCK : (og + 1) * OGRP * BLOCK,
                        ],
                        in_=osb_t[:],
                    )

    nc.compile()
    return nc


def prep_inputs(x, S, U, Vt, bias):
    """Host-side layout prep. Returns per-core input maps."""
    x = np.ascontiguousarray(np.asarray(x, dtype=np.float32))
    S = np.asarray(S, dtype=np.float32)
    U = np.asarray(U, dtype=np.float32)
    Vt = np.asarray(Vt, dtype=np.float32)
    bias = np.asarray(bias, dtype=np.float32)

    xt = np.ascontiguousarray(x.reshape(TOK, IN_DIM).T)  # (4096, 2048)

    vt_aug = np.zeros((B_IN, BLOCK, CP), np.float32)
    vt_aug[:, :, :RANK] = Vt
    vt_aug[:, :, RANK] = 1.0  # rowsum column

    # s_flat[0, i*CAP + o*CP + r] = S_aug[o, i, r]; pad r>=17 stays 0
    s_pad = np.zeros((B_IN, B_OUT, CP), np.float32)
    s_pad[:, :, :RANK] = S.transpose(1, 0, 2)
    s_pad[:, :, RANK] = 1.0  # rowsum column weight
    s_flat = np.ascontiguousarray(s_pad.reshape(1, B_IN * CAP))

    # row 16 multiplies mid row (rowsum+1) -> bias;  row 17 multiplies the
    # constant 1.0 padding row and cancels the +1 bank-init pollution of the
    # 16 rank rows: -sum_r U[o,r,:]
    bias_row = bias.reshape(B_OUT, 1, BLOCK)
    comp_row = -U.sum(axis=1, keepdims=True)  # (16, 1, 256)
    u_aug = np.ascontiguousarray(
        np.concatenate([U, bias_row, comp_row], axis=1)
    )  # (16, 18, 256)

    # shipped A chunk pairs (even i): A[(i,p),(o,r)] = vt_aug[i,p,r]*s_pad[i,o,r]
    a_even = np.einsum(
        "ipr,ior->ipor", vt_aug[0::2], s_pad[0::2]
    )  # (8, 256, 16, 32)
    aship = np.ascontiguousarray(a_even.reshape(B_IN // 2, 2 * 128, CAP))

    rng = np.random.default_rng(0)
    wseed = rng.standard_normal((128, BLOCK), dtype=np.float32)

    konst = np.zeros((1, 2 * TPC), np.float32)
    konst[0, :TPC] = 1.0

    in_maps = []
    for c in range(N_CORES):
        in_maps.append(
            {
                "xt": np.ascontiguousarray(xt[:, c * TPC : (c + 1) * TPC]),
                "vt": vt_aug,
                "s_flat": s_flat,
                "aship": aship,
                "u_mat": u_aug,
                "wseed": wseed,
                "konst": konst,
            }
        )
    return in_maps


def kernel(x, S, U, Vt, bias):
    global LAST_RESULTS
    from concourse.bass_utils import run_bass_kernel_spmd

    if "nc" not in _CACHE:
        _CACHE["nc"] = build_program()
    nc = _CACHE["nc"]

    in_maps = prep_inputs(x, S, U, Vt, bias)
    res = run_bass_kernel_spmd(
        nc, in_maps, list(range(N_CORES)), trace=TRACE, tmpdir=TRACE_DIR
    )
    LAST_RESULTS = res
    out = np.concatenate([res.results[c]["out"] for c in range(N_CORES)], axis=0)
    return out.reshape(2, TOK // 2, OUT_DIM)



# revision 13
# speedup vs baseline: 1.4191x; 1.4191x over previous
"""Trainium2 Bass kernel for nn_Blast: out = x @ (W0 + 1 bias^T) + bias
where W0 block (i_in, i_out) = Vt[i] @ diag(S[o,i]) @ U[o].

v2: full bf16 dataflow (tolerance is 2e-2; bf16 end-to-end lands ~5e-3).

Per core (256 tokens):
  midT[(o,r), tok] = sum_in A[in, (o,r)] * xT[in, tok]     (A = Vt*S, built on device)
  out[tok, oq]     = sum_r midT[(o,r), tok] * U'[o, r, q]

Mid rows pack o-blocks as bank = o%3, slot = o//3: 16 rank rows per slot plus
a shared rowsum row (112; A ones-column -> bias*(rowsum+1) supplies both bias
terms) and a comp row (113; cancels the +1.0 bank-open pollution): bank width
W=114, 3 PSUM banks. Phase A = 32 K-chunks x 3 matmuls (N=256, M=114).

Phase B reads each mid bank wholesale (K=114) against the sparse stacked-U
matrix usb[16s+r, 256o+q] = U[o,r,q] iff s==o//3: one matmul per o-block
(N=256), paired two per PSUM tile so PSUM->SBUF copies run at [128,512].
The mod-3 bank map makes each usb 16-row slot group cover contiguous
o-blocks, so U loads with 6 plain DMAs - no on-device scatter.

DMA (shared 16-SDMA pool, ~358 GB/s/core): x 2MB on sync ring; consts
(vt/s/uc/bc) on scalar ring; aship on gpsimd; out 2MB split sync/gpsimd.
A is built on device (DVE/GPS muls of Vt against PE-staged S rows read
straight from PSUM); first KSHIP chunks ship prebuilt to cover the ramp.
"""

import numpy as np

IN_DIM = 4096
OUT_DIM = 4096
BLOCK = 256
RANK = 16
B_IN = 16
B_OUT = 16
N_CORES = 8
TOK = 2048
TPC = TOK // N_CORES          # 256 tokens per core
NCHUNK = IN_DIM // 128        # 32 K-chunks
W = 114                       # mid-bank width: 7*16 rank + rowsum + comp
NB = 3                        # mid banks, o-block -> bank o%3 slot o//3
SW = NB * W                   # 342 A-columns per chunk
KSHIP = 8                     # prebuilt A chunks shipped from host
NWARM = 20                    # PE warm matmuls before staging
XBATCH = [4, 4, 8, 8, 8]      # x chunk batching per DMA

_CACHE = {}

# test.py toggles; harness never touches these
TRACE = False
TRACE_DIR = None
LAST_RESULTS = None


def _bank_slot(o):
    return o % 3, o // 3


def build_program():
    import concourse.mybir as mybir
    from concourse import bacc
    from concourse.tile import TileContext

    bf16 = mybir.dt.bfloat16
    f32 = mybir.dt.float32

    nc = bacc.Bacc(trn_type="TRN2")
    xt_d = nc.dram_tensor("xt", (IN_DIM, TPC), bf16, kind="ExternalInput")
    vt_d = nc.dram_tensor("vt", (128, NCHUNK * RANK), bf16, kind="ExternalInput")
    s_d = nc.dram_tensor("s_flat", (1, B_IN * SW), bf16, kind="ExternalInput")
    uc_d = nc.dram_tensor("uc", (96, 3 * BLOCK), bf16, kind="ExternalInput")
    bc_d = nc.dram_tensor("bc", (2, OUT_DIM), bf16, kind="ExternalInput")
    aship_d = nc.dram_tensor("aship", (128, KSHIP * SW), bf16, kind="ExternalInput")
    out_d = nc.dram_tensor("out", (TPC, OUT_DIM), bf16, kind="ExternalOutput")

    with TileContext(nc) as tc:
        from contextlib import ExitStack

        with ExitStack() as ctx:
            consts = ctx.enter_context(tc.tile_pool(name="consts", bufs=1))
            xpool = ctx.enter_context(tc.tile_pool(name="xpool", bufs=1))
            apool = ctx.enter_context(tc.tile_pool(name="apool", bufs=1))
            midsb = ctx.enter_context(tc.tile_pool(name="midsb", bufs=1))
            outsb = ctx.enter_context(tc.tile_pool(name="outsb", bufs=1))
            ps_mid = ctx.enter_context(
                tc.tile_pool(name="ps_mid", bufs=1, space="PSUM")
            )

            # ---- constants / memsets (no DMA deps) ----
            ones_sb = consts.tile([1, TPC], bf16, name="ones_sb", tag="ones_sb")
            nc.vector.memset(ones_sb[:], 1.0)

            usb = consts.tile([128, OUT_DIM], bf16, name="usb", tag="usb")
            nc.vector.memset(usb[0:112, :], 0.0)

            # A storage: [128 xrows, chunk k, bank b, col w]
            a_all = apool.tile([128, NCHUNK * SW], bf16, name="a_all", tag="a_all")
            a_v = a_all[:].rearrange("p (k b w) -> p k b w", k=NCHUNK, w=W)
            # built chunks: ones col 112, zero col 113 (rank cols come from muls)
            nc.vector.memset(a_v[:, KSHIP:NCHUNK, :, 112:113], 1.0)
            nc.vector.memset(a_v[:, KSHIP:NCHUNK, :, 113:114], 0.0)

            # ---- input DMAs ----
            # scalar ring: vt + s (gate A builds), then uc + bc (phase B)
            vt_sb = consts.tile([128, NCHUNK * RANK], bf16, name="vt_sb", tag="vt_sb")
            nc.scalar.dma_start(out=vt_sb[:], in_=vt_d[:])
            s_sb = consts.tile([1, B_IN * SW], bf16, name="s_sb", tag="s_sb")
            nc.scalar.dma_start(out=s_sb[:], in_=s_d[:])
            # (uc/bc doorbells are emitted after the build loop so the
            # scalar engine runs the gpsimd-feeding stage copies first)

            # gpsimd ring: shipped A head
            nc.gpsimd.dma_start(out=a_all[:, 0 : KSHIP * SW], in_=aship_d[:])

            # sync ring: x batches
            xbatches = []
            xoff = []
            k0 = 0
            for nk in XBATCH:
                xb = xpool.tile([128, nk * TPC], bf16, name=f"xb{k0}", tag=f"xb{k0}")
                nc.sync.dma_start(
                    out=xb[:].rearrange("p (k t) -> p k t", k=nk),
                    in_=xt_d[k0 * 128 : (k0 + nk) * 128, :].rearrange(
                        "(k p) t -> p k t", p=128
                    ),
                )
                xbatches.append(xb)
                xoff.append(k0)
                k0 += nk

            def xchunk(k):
                for xb, o in zip(xbatches, xoff):
                    nk = xb.shape[1] // TPC
                    if o <= k < o + nk:
                        return xb[:, (k - o) * TPC : (k - o + 1) * TPC]
                raise AssertionError

            mids = []
            for b in range(NB):
                ms = midsb.tile([128, TPC], bf16, name=f"mids{b}", tag=f"mids{b}")
                mids.append(ms)

            with tc.tile_pool(name="ps_pre", bufs=1, space="PSUM") as ps_pre:
                # ---- PE warmup (no DMA deps: ones via memset) ----
                warm = ps_pre.tile([128, TPC], f32, name="warm", tag="warm", bufs=1)
                for _ in range(NWARM):
                    nc.tensor.matmul(
                        warm[:],
                        lhsT=ones_sb[0:1, 0:128],
                        rhs=ones_sb[:],
                        start=True,
                        stop=True,
                        tile_position=(0, 0),
                    )

                # ---- stage S rows to 128 partitions (PE broadcast), build A
                # chunks with DVE/GPS muls reading the stage PSUM directly ----
                def ap_bsr(t):
                    return (
                        t.rearrange("p (b w) -> p b w", b=NB)[:, :, 0:112]
                        .rearrange("p b (s r) -> p b s r", r=RANK)
                    )

                for i in range(B_IN):
                    ks = [k for k in (2 * i, 2 * i + 1) if k >= KSHIP]
                    if not ks:
                        continue
                    sp = ps_pre.tile([128, SW], f32, name="sp", tag="sp", bufs=3)
                    nc.tensor.matmul(
                        sp[:],
                        lhsT=ones_sb[0:1, 0:128],
                        rhs=s_sb[0:1, i * SW : (i + 1) * SW],
                        start=True,
                        stop=True,
                        tile_position=(0, 0),
                    )
                    # gpsimd can't read PSUM: give its builds an SBUF copy
                    gks = [k for k in ks if k % 4 == 1]
                    if gks:
                        sps = consts.tile([128, SW], bf16, name=f"sps{i}",
                                          tag=f"sps{i}")
                        nc.scalar.copy(sps[:], sp[:])
                    for k in ks:
                        if k % 4 == 1:
                            nc.gpsimd.tensor_mul(
                                a_v[:, k, :, 0:112].rearrange(
                                    "p b (s r) -> p b s r", r=RANK
                                ),
                                vt_sb[:, RANK * k : RANK * (k + 1)]
                                .unsqueeze(1)
                                .unsqueeze(2)
                                .broadcast_to([128, NB, 7, RANK]),
                                ap_bsr(sps[:]),
                            )
                        else:
                            nc.vector.tensor_mul(
                                a_v[:, k, :, 0:112].rearrange(
                                    "p b (s r) -> p b s r", r=RANK
                                ),
                                vt_sb[:, RANK * k : RANK * (k + 1)]
                                .unsqueeze(1)
                                .unsqueeze(2)
                                .broadcast_to([128, NB, 7, RANK]),
                                ap_bsr(sp[:]),
                            )

                # scalar ring: uc + bc doorbells (queued after stage copies)
                for s in range(6):
                    n_o = 3 if s < 5 else 1
                    nc.scalar.dma_start(
                        out=usb[16 * s : 16 * s + 16, 768 * s : 768 * s + 256 * n_o],
                        in_=uc_d[16 * s : 16 * s + 16, 0 : 256 * n_o],
                    )
                nc.scalar.dma_start(out=usb[112:114, :], in_=bc_d[:])

                # ---- open mid banks with 1.0 everywhere ----
                midp = []
                for b in range(NB):
                    mp = ps_mid.tile([128, TPC], f32, name=f"midp{b}", tag=f"midp{b}")
                    nc.tensor.matmul(
                        mp[0:W, :],
                        lhsT=ones_sb[0:1, 0:W],
                        rhs=ones_sb[:],
                        start=True,
                        stop=False,
                        tile_position=(0, 0),
                    )
                    midp.append(mp)

                # ---- phase A: midT accumulation over 32 K-chunks ----
                for k in range(NCHUNK):
                    for b in range(NB):
                        nc.tensor.matmul(
                            midp[b][0:W, :],
                            lhsT=a_v[:, k, b, :],
                            rhs=xchunk(k),
                            start=False,
                            stop=(k == NCHUNK - 1),
                            tile_position=(0, 0),
                        )

                # ---- midT -> SBUF (bf16 cast), token halves for pipelining ----
                for tt in range(2):
                    for b in range(NB):
                        sl = (slice(0, W), slice(tt * 128, (tt + 1) * 128))
                        if (b + tt) % 2 == 0:
                            nc.vector.tensor_copy(mids[b][sl], midp[b][sl])
                        else:
                            nc.scalar.copy(mids[b][sl], midp[b][sl])

            # ---- phase B: per o-block K=114 matmuls (N=256), paired into
            # [128,512] PSUM tiles; flush 2048-col halves as they land ----
            ps_out = ctx.enter_context(
                tc.tile_pool(name="ps_out", bufs=4, space="PSUM")
            )
            for tt in range(2):
                osb_t = outsb.tile(
                    [128, OUT_DIM], bf16, name=f"osb{tt}", tag=f"osb{tt}"
                )
                for j in range(8):
                    po = ps_out.tile([128, 512], f32, name="po", tag="po")
                    for oo in (2 * j, 2 * j + 1):
                        b, _ = _bank_slot(oo)
                        nc.tensor.matmul(
                            po[:, (oo % 2) * BLOCK : (oo % 2 + 1) * BLOCK],
                            lhsT=mids[b][0:W, tt * 128 : (tt + 1) * 128],
                            rhs=usb[0:W, BLOCK * oo : BLOCK * (oo + 1)],
                            start=True,
                            stop=True,
                            tile_position=(0, 0),
                        )
                    if j % 2 == 0:
                        nc.vector.tensor_copy(
                            osb_t[:, 512 * j : 512 * (j + 1)], po[:]
                        )
                    else:
                        nc.scalar.copy(
                            osb_t[:, 512 * j : 512 * (j + 1)], po[:]
                        )
                    if j == 3:
                        nc.sync.dma_start(
                            out=out_d[tt * 128 : (tt + 1) * 128, 0:2048],
                            in_=osb_t[:, 0:2048],
                        )
                    elif j == 7:
                        nc.gpsimd.dma_start(
                            out=out_d[tt * 128 : (tt + 1) * 128, 2048:4096],
                            in_=osb_t[:, 2048:4096],
                        )

    nc.compile()
    return nc


def prep_inputs(x, S, U, Vt, bias):
    """Host-side layout prep (bf16). Returns per-core input maps."""
    import ml_dtypes

    bf = ml_dtypes.bfloat16
    x = np.asarray(x, dtype=np.float32)
    S = np.asarray(S, dtype=np.float32)
    U = np.asarray(U, dtype=np.float32)
    Vt = np.asarray(Vt, dtype=np.float32)
    bias = np.asarray(bias, dtype=np.float32)

    xt = np.ascontiguousarray(x.reshape(TOK, IN_DIM).T).astype(bf)  # (4096, 2048)

    # vt[p, 16k+r] = Vt[i, 128h+p, r], k = 2i+h
    vt_host = np.ascontiguousarray(
        Vt.reshape(B_IN * 2, 128, RANK).transpose(1, 0, 2).reshape(128, NCHUNK * RANK)
    ).astype(bf)

    # s_flat[(i, b, s, r)] = S[o(b,s), i, r]; zero where no o-block
    s_pack = np.zeros((B_IN, NB, W), np.float32)
    for o in range(B_OUT):
        b, s = _bank_slot(o)
        s_pack[:, b, 16 * s : 16 * s + 16] = S[o, :, :]
    s_flat = s_pack.reshape(1, B_IN * NB * W).astype(bf)

    # uc[16s+r, 256j+q] = U[3s+j, r, q] (row-group s covers o = 3s..3s+2)
    uc = np.zeros((96, 3 * BLOCK), np.float32)
    for o in range(B_OUT):
        s, j = o // 3, o % 3
        uc[16 * s : 16 * s + 16, BLOCK * j : BLOCK * (j + 1)] = U[o]
    uc = uc.astype(bf)

    # bc row0 = bias (x rowsum row), row1 = comp = -sum_r U[o]
    bc = np.stack([bias, -U.sum(axis=1).reshape(-1)]).astype(bf)

    # shipped A head: chunks 0..KSHIP-1 in [p, k, b, w] layout
    aship = np.zeros((128, KSHIP, NB, W), np.float32)
    for k in range(KSHIP):
        i, h = k // 2, k % 2
        vt_k = Vt[i, 128 * h : 128 * (h + 1), :]  # [128, 16]
        for o in range(B_OUT):
            b, s = _bank_slot(o)
            aship[:, k, b, 16 * s : 16 * s + 16] = vt_k * S[o, i, :][None, :]
        aship[:, k, :, 112] = 1.0
    aship = aship.reshape(128, KSHIP * SW).astype(bf)

    in_maps = []
    for c in range(N_CORES):
        in_maps.append(
            {
                "xt": np.ascontiguousarray(xt[:, c * TPC : (c + 1) * TPC]),
                "vt": vt_host,
                "s_flat": s_flat,
                "uc": uc,
                "bc": bc,
                "aship": aship,
            }
        )
    return in_maps


def kernel(x, S, U, Vt, bias):
    global LAST_RESULTS
    from concourse.bass_utils import run_bass_kernel_spmd

    if "nc" not in _CACHE:
        _CACHE["nc"] = build_program()
    nc = _CACHE["nc"]

    in_maps = prep_inputs(x, S, U, Vt, bias)
    res = run_bass_kernel_spmd(
        nc, in_maps, list(range(N_CORES)), trace=TRACE, tmpdir=TRACE_DIR
    )
    LAST_RESULTS = res
    out = np.concatenate(
        [np.asarray(res.results[c]["out"]).astype(np.float32) for c in range(N_CORES)],
        axis=0,
    )
    return out.reshape(2, TOK // 2, OUT_DIM)
